# revision 1
# baseline (speedup 1.0000x reference)
"""Linear-chain CRF partition function (log Z) on 8 Trainium2 NeuronCores.

Strategy: the per-step logsumexp over 'from' tags is rewritten in the exp
domain as a matmul with the fixed matrix exp(trans).T, so each time step is
one 128x128x256 PE matmul followed by one elementwise multiply with
exp(feat_t - 5) on DVE.  The sequential 1024-step scan is split into 24 time
segments (3 per core); every segment processes ALL 256 batch lanes per step
([128, 256] tiles amortize the fixed instruction overheads).  Segments j>=1
start from a uniform vector and run a short redundant warmup: the positive
transition matrix contracts direction errors by ~50x per step (measured), so
a handful of warmup steps converge the state to the true forward direction
far below the bf16 noise floor.  Per-sequence scales are stitched across
segments via colsum ratios:

  logZ = ln(w . y_last) + sum_{j<last} ln(colsum y_j)
         - sum_{j>=1} ln(colsum d_j) + 5 * S

where y_j is segment j's final state and d_j its state at the segment start.
The logs are taken on the host from the raw DMA'd sums.  No per-step
renormalization is needed: within one 46-step chain the state stays inside
f32/bf16 exponent range.
"""

import numpy as np
import ml_dtypes

import concourse.bacc as bacc
import concourse.bass as bass
import concourse.tile as tile
from concourse import mybir
from concourse._compat import with_exitstack
from concourse.bass_utils import run_bass_kernel_spmd

B, S, T2 = 256, 1024, 128
NCORES = 8
CPC = 3                 # chains (time segments) per core
NCH = NCORES * CPC      # 24
NSLOT = 46              # steps per chain
WARMUPS = [7, 2, 2]     # warmup steps by chain position (chain 0: all real)
# coverage: 46 + 7*(46-7) + 8*(46-2) + 8*(46-2) = 1023 real steps
# feature-chunk step counts: ramped so compute starts early while staying
# ahead of the globally-shared DMA bandwidth
CHUNKS = [2, 4, 8, 16, 16]
assert sum(CHUNKS) == NSLOT
START, END = T2 - 1, T2 - 2
SHIFT = 5.0
BF16, F32 = mybir.dt.bfloat16, mybir.dt.float32
NPBF = ml_dtypes.bfloat16


def _starts():
    # segment j = CPC*k + i (core k, position i); real windows tile [1, 1024)
    R = [1]
    for j in range(1, NCH):
        prev_len = NSLOT if j - 1 == 0 else NSLOT - WARMUPS[(j - 1) % CPC]
        R.append(R[-1] + prev_len)
    assert R[-1] + (NSLOT - WARMUPS[(NCH - 1) % CPC]) == S
    return [R[j] - (0 if j == 0 else WARMUPS[j % CPC]) for j in range(NCH)]


STARTS = _starts()


@with_exitstack
def _body(ctx, tc, OUT_d, CT_d, F_d):
    nc = tc.nc
    const = ctx.enter_context(tc.tile_pool(name="const", bufs=1))
    fpool = ctx.enter_context(tc.tile_pool(name="f", bufs=3))
    ppool = ctx.enter_context(tc.tile_pool(name="p", bufs=3))
    qpool = ctx.enter_context(
        tc.tile_pool(name="q", bufs=2, space=bass.MemorySpace.PSUM)
    )
    smpool = ctx.enter_context(
        tc.tile_pool(name="sm", bufs=2, space=bass.MemorySpace.PSUM)
    )
    # one DMA-issuing engine per chain so the chains' feature streams don't
    # serialize behind each other's descriptors; consts go on a fourth queue
    dma_eng = [nc.sync, nc.gpsimd, nc.scalar]

    fts = [None] * CPC
    bounds = list(np.cumsum([0] + CHUNKS))[:-1]

    # all constants arrive in one DMA: [ET | GE | PINIT0..2] along the free dim
    cw = T2 + 2 + CPC * B
    ct = const.tile([T2, cw], BF16, tag="consts")
    nc.sync.dma_start(ct[:], CT_d[:])
    et = ct[:, 0:T2]
    ge = ct[:, T2 : T2 + 2]  # col0 = ones, col1 = exp(trans[END])
    p = [ct[:, T2 + 2 + i * B : T2 + 2 + (i + 1) * B] for i in range(CPC)]

    # first feature chunks next: they gate the first multiplies
    for i in range(CPC):
        ft = fpool.tile([T2, CHUNKS[0], B], BF16, tag=f"fch{i}")
        dma_eng[i].dma_start(ft[:], F_d[i][:, 0 : CHUNKS[0], :])
        fts[i] = ft

    def sums_out(i, pp, row0, nrows):
        # [colsum(p); w.p] -> OUT rows [row0 : row0+nrows] (logs taken on host)
        sm = smpool.tile([2, B], F32, tag="sm")
        nc.tensor.matmul(sm[:], ge[:], pp[:], start=True, stop=True)
        cp = ppool.tile([2, B], F32, tag="cp")
        nc.scalar.copy(cp[0:nrows, :], sm[0:nrows, :])  # ACT is otherwise idle
        dma_eng[i].dma_start(OUT_d[row0 : row0 + nrows, :], cp[0:nrows, :])

    for s in range(NSLOT):
        if s in bounds:
            ci = bounds.index(s)
            if ci > 0:
                cs = CHUNKS[ci]
                for i in range(CPC):
                    ft = fpool.tile([T2, cs, B], BF16, tag=f"fch{i}")
                    dma_eng[i].dma_start(ft[:], F_d[i][:, s : s + cs, :])
                    fts[i] = ft
            coff = 0
        for i in range(CPC):
            if s == WARMUPS[i]:
                sums_out(i, p[i], 3 * i, 1)  # delta_j colsum
            q = qpool.tile([T2, B], F32, tag=f"q{i}")
            nc.tensor.matmul(q[:], et[:], p[i][:], start=True, stop=True)
            pn = ppool.tile([T2, B], BF16, tag=f"p{i}")
            nc.vector.tensor_mul(pn[:], q[:], fts[i][:, coff, :])
            p[i] = pn
        coff += 1
    for i in range(CPC):
        sums_out(i, p[i], 3 * i + 1, 2)  # [gamma_j; w.y_j]


_NC_CACHE = {}


def _get_nc():
    if "nc" not in _NC_CACHE:
        nc = bacc.Bacc("TRN2", target_bir_lowering=False, debug=False)
        CT_d = nc.dram_tensor(
            "CT", [T2, T2 + 2 + CPC * B], BF16, kind="ExternalInput"
        )
        F_d = [
            nc.dram_tensor(f"F{i}", [T2, NSLOT, B], BF16, kind="ExternalInput")
            for i in range(CPC)
        ]
        OUT_d = nc.dram_tensor("OUT", [3 * CPC, B], F32, kind="ExternalOutput")
        with tile.TileContext(nc) as tc:
            _body(tc, OUT_d, CT_d, F_d)
        nc.compile()
        _NC_CACHE["nc"] = nc
    return _NC_CACHE["nc"]


def prepare_in_maps(feats, trans):
    feats = np.asarray(feats, dtype=np.float32)
    trans = np.asarray(trans, dtype=np.float32)
    assert feats.shape == (B, S, T2) and trans.shape == (T2, T2)

    with np.errstate(under="ignore"):
        ET = np.exp(trans).T  # [from, to]
        GE = np.ones((T2, 2), np.float32)
        GE[:, 1] = np.exp(trans[END, :])
        p0 = np.exp(trans[:, START])[:, None] * np.exp(
            feats[:, 0, :].T - SHIFT
        )  # [T2, B]
        fexp = np.exp(feats - SHIFT).astype(NPBF)  # [B, S, T2]
    F_full = np.ascontiguousarray(fexp.transpose(2, 1, 0))  # [T2, S, B]

    # constants blob: [ET | GE | PINIT0..2]; PINIT j=0 is the exact CRF init,
    # warmup chains start from ones
    CT = np.ones((NCORES, T2, T2 + 2 + CPC * B), np.float32)
    CT[:, :, 0:T2] = ET
    CT[:, :, T2 : T2 + 2] = GE
    CT[0, :, T2 + 2 : T2 + 2 + B] = p0
    CT = CT.astype(NPBF)

    in_maps = []
    for k in range(NCORES):
        m = {"CT": CT[k]}
        for i in range(CPC):
            t0 = STARTS[CPC * k + i]
            m[f"F{i}"] = np.ascontiguousarray(F_full[:, t0 : t0 + NSLOT, :])
        in_maps.append(m)
    return in_maps


def postprocess(results):
    # OUT[3*CPC, B] per core: row 3i = delta colsum, 3i+1 = gamma colsum,
    # 3i+2 = w . y  (raw sums; logs taken here)
    logZ = np.zeros(B, dtype=np.float64)
    for k, r in enumerate(results):
        out = r["OUT"].astype(np.float64)
        for i in range(CPC):
            j = CPC * k + i
            if j == NCH - 1:
                logZ += np.log(out[3 * i + 2])
            else:
                logZ += np.log(out[3 * i + 1])
            if j >= 1:
                logZ -= np.log(out[3 * i])
    logZ += SHIFT * S
    return logZ.astype(np.float32)


def run(feats, trans, trace=False, **spmd_kwargs):
    nc = _get_nc()
    in_maps = prepare_in_maps(feats, trans)
    res = run_bass_kernel_spmd(
        nc, in_maps, list(range(NCORES)), trace=trace, **spmd_kwargs
    )
    return postprocess(res.results), res


def kernel(feats, trans):
    out, _ = run(feats, trans, trace=False)
    return out



# revision 2
# speedup vs baseline: 1.0182x; 1.0182x over previous
"""Linear-chain CRF partition function (log Z) on 8 Trainium2 NeuronCores.

Exp-domain formulation: each step is q = W @ p (PE matmul, W = exp(trans-6).T)
followed by an elementwise multiply with f_t = exp(feat_t + 1).  The net
per-step shift is 5, accounted on the host (logZ += 5*S).

Time is split into 8 windows per batch column per core (64 windows of ~16
real steps per column globally).  All windows of a core advance in lockstep
through R=18 slots: w warmup slots (converge direction from a uniform start;
the transition matrix contracts direction error ~50x/step) then real steps.
Window scales are stitched on the host via colsum ratios (delta at the first
real slot, gamma at the end).

The 2048 in-flight columns (8 windows x 256 batch) are split across three
engine pipelines so every engine contributes:
  A (874 cols, bf16 feats): PE -> ACT copy (PSUM f32 -> SBUF bf16)
                            -> DVE 2x multiply (all-bf16).
  D (690 cols, fp8 feats):  PE -> DVE multiply (PSUM f32 x fp8).
  Z (484 cols, fp8 feats):  PE -> Pool (gpsimd) multiply.
Each lane owns its PSUM tile (concurrent readers need distinct tiles); the
stitching colsums reuse rows 0-1 of the lane's own PSUM tile so the four
drain pipelines run in parallel.
"""

import numpy as np
import ml_dtypes

import concourse.bacc as bacc
import concourse.bass as bass
import concourse.tile as tile
from concourse import mybir
from concourse._compat import with_exitstack
from concourse.bass_utils import run_bass_kernel_spmd

B, S, T2 = 256, 1024, 128
NCORES = 8
START, END = T2 - 1, T2 - 2
SHIFT_W = 6.0    # folded into the transition weights
SHIFT_F = -1.0   # folded into the features (f = exp(feat + 1))
NET_SHIFT = SHIFT_W + SHIFT_F  # 5 per step, host-corrected

R = 17                      # slots per window
NWIN = 8                    # windows per batch column per core
F_COLS = NWIN * B           # 2048 in-flight columns per core
CHUNKS = [2, 2, 3, 3, 4, 3]  # feature-DMA chunking over the R slots
assert sum(CHUNKS) == R

# lanes: (name, path, lo, hi) over the 2048 column axis
import os
_SPLIT = os.environ.get("LANE_SPLIT", "437,437,662,512")
A1, A2, DC, ZC = (int(x) for x in _SPLIT.split(","))
assert A1 + A2 + DC + ZC == F_COLS
LANES = [
    ("d", "D", A1 + A2, A1 + A2 + DC),
    ("a1", "A", 0, A1),
    ("a2", "A", A1, A1 + A2),
    ("z", "Z", A1 + A2 + DC, F_COLS),
]

# Per-core spans: core 0 has the exact-init window (18 real steps, w=0),
# core 7 ends with a 13-real window (w=5).  All other windows: 16 real, w=2.
# Column c of a core: window wi = c // 256, batch b = c % 256.
# All windows end their real span at slot R-1; warmup count w = R - len.
CORE_START = [1, 131, 259, 387, 515, 643, 771, 899]


def _win_table():
    """[core][wi] -> (t_first_real, w).  Slot s covers t = t_first_real-w+s."""
    tab = []
    for k in range(NCORES):
        rows = []
        t = CORE_START[k]
        for wi in range(NWIN):
            if k == 0 and wi == 0:
                ln, w = 17, 0
            elif k == 7 and wi == NWIN - 1:
                ln, w = 13, 4
            else:
                ln, w = 16, 1
            rows.append((t, w))
            t += ln
        tab.append(rows)
    assert t == S, (k, t)
    return tab


WTAB = _win_table()

BF16, F32, FP8 = mybir.dt.bfloat16, mybir.dt.float32, mybir.dt.float8e4
NPBF = ml_dtypes.bfloat16
NPF8 = ml_dtypes.float8_e4m3

SUM_ENG = {"a1": "scalar", "a2": "scalar", "d": "scalar", "z": "gpsimd"}
SUM_ENG_G = {"a1": "scalar", "a2": "vector", "d": "vector", "z": "scalar"}


@with_exitstack
def _body(ctx, tc, OUT_d, CT_d, F_d):
    nc = tc.nc
    const = ctx.enter_context(tc.tile_pool(name="const", bufs=1))
    ppool = ctx.enter_context(tc.tile_pool(name="p", bufs=4))
    fpool = ctx.enter_context(tc.tile_pool(name="f", bufs=1))
    qcpool = ctx.enter_context(tc.tile_pool(name="qc", bufs=2))
    opool = ctx.enter_context(tc.tile_pool(name="o", bufs=6))
    qpool = ctx.enter_context(
        tc.tile_pool(name="q", bufs=1, space=bass.MemorySpace.PSUM)
    )
    smpool = ctx.enter_context(
        tc.tile_pool(name="sm", bufs=1, space=bass.MemorySpace.PSUM)
    )
    sm_tiles = {}

    ct = const.tile([T2, T2 + 2 + B], BF16, tag="ct")
    nc.sync.dma_start(ct[:], CT_d[:])
    wmat = ct[:, 0:T2]
    ge = ct[:, T2 : T2 + 2]  # col0 = ones, col1 = exp(trans[END])

    # state init: ones everywhere (memset), exact p0 over core0-window0 (DMA;
    # other cores receive ones there too, data-driven)
    ps, qs, fts = {}, {}, {}
    meng = {"a1": nc.vector, "a2": nc.vector, "d": nc.vector, "z": nc.gpsimd}
    for name, path, lo, hi in LANES:
        n = hi - lo
        p = ppool.tile([T2, n], BF16, tag=f"p{name}", name=f"p_{name}_init")
        meng[name].memset(p[:], 1.0)
        ps[name] = p
        qs[name] = qpool.tile([T2, n], F32, tag=f"q{name}", name=f"q_{name}")
    nc.vector.tensor_copy(ps["a1"][:, 0:B], ct[:, T2 + 2 : T2 + 2 + B])

    bounds = list(np.cumsum([0] + CHUNKS))[:-1]
    # every chunk gets its own tile; all feature DMAs issue up front in need
    # order, so the serial DMA stream runs ahead of compute throughout
    chunk_tiles = {name: {} for name, _, _, _ in LANES}
    for ci in range(len(CHUNKS)):
        for name, path, lo, hi in LANES:
            n = hi - lo
            cs, b0 = CHUNKS[ci], bounds[ci]
            ft = fpool.tile([T2, cs, n], BF16 if path == "A" else FP8,
                            tag=f"f{name}c{ci}", name=f"f_{name}_c{ci}")
            nc.sync.dma_start(ft[:], F_d[name][:, b0 : b0 + cs, :])
            chunk_tiles[name][ci] = ft
    fts = {name: chunk_tiles[name][0] for name, _, _, _ in LANES}
    issue_at = {}

    out_dmas = []

    def sums(row, lanes_cols, tag, src, sm_slot=None):
        # colsum rows of src[name] into scratch (or the lane's own q tile for
        # the final gammas), copy to SBUF, DMA out via the SP queue.
        for name, lo, hi, nr in lanes_cols:
            n = hi - lo
            if tag == "g":
                q = qs[name]
            else:
                if sm_slot not in sm_tiles:
                    sm_tiles[sm_slot] = smpool.tile(
                        [1, max(DC, A1) if sm_slot == 0 else max(ZC, A2)], F32,
                        tag=f"sm{sm_slot}", name=f"sm{sm_slot}")
                q = sm_tiles[sm_slot][:, 0:n]
            nc.tensor.matmul(q[0:nr, :], ge[:, 0:nr], src[name][:], start=True,
                             stop=True)
            cp = opool.tile([2, n], F32, tag="cp", name=f"cp_{tag}_{name}")
            eng = getattr(nc, (SUM_ENG_G if tag == "g" else SUM_ENG)[name])
            if eng is nc.scalar:
                eng.copy(cp[0:nr, :], q[0:nr, :])
            else:
                eng.tensor_copy(cp[0:nr, :], q[0:nr, :])
            if tag == "g":
                nc.sync.dma_start(OUT_d[row : row + nr, lo:hi], cp[0:nr, :])
            else:
                out_dmas.append((row, nr, lo, hi, cp))

    def lane_step(name, path, lo, hi, ls):
        n = hi - lo
        ci = max(i for i, b in enumerate(bounds) if b <= ls)
        f = chunk_tiles[name][ci][:, ls - bounds[ci], :]
        q, p = qs[name], ps[name]
        for c0 in range(0, n, 512):
            c1 = min(c0 + 512, n)
            nc.tensor.matmul(q[:, c0:c1], wmat[:], p[:, c0:c1], start=True,
                             stop=True)
        pn = ppool.tile([T2, n], BF16, tag=f"p{name}", name=f"p_{name}_{ls}")
        if path == "A":
            qc = qcpool.tile([T2, n], BF16, tag=f"qc{name}",
                             name=f"qc_{name}_{ls}")
            nc.scalar.copy(qc[:], q[:])
            nc.vector.tensor_mul(pn[:], qc[:], f[:])
        elif path == "D":
            nc.vector.tensor_mul(pn[:], q[:], f[:])
        else:
            nc.gpsimd.tensor_mul(pn[:], q[:], f[:])
        ps[name] = pn

    for s in range(R + 1):
        if s == 1:
            state_s1 = dict(ps)
            sums(0, [("a1", 0, A1, 1)], "d2", state_s1, sm_slot=0)
        if s == 2:
            # z runs a round behind: live ps["z"] is its post-step-0 state
            sums(0, [("z", A1 + A2 + DC, F_COLS, 1)], "d2", ps, sm_slot=1)
        if s == 3:
            sums(0, [("d", A1 + A2, A1 + A2 + DC, 1)], "d2", state_s1, sm_slot=0)
        if s == 4:
            sums(0, [("a2", A1, A1 + A2, 1)], "d2", state_s1, sm_slot=1)
        if s == 5:
            # z post-step-3 state (w=4 window)
            sums(1, [("z", A1 + A2 + DC, F_COLS, 1)], "d5", ps, sm_slot=0)
        if s == 8:
            for row, nr, lo, hi, cp in out_dmas:
                nc.sync.dma_start(OUT_d[row : row + nr, lo:hi], cp[0:nr, :])
            out_dmas.clear()
        for name, path, lo, hi in LANES:
            if name == "z":
                if s >= 1:
                    lane_step(name, path, lo, hi, s - 1)
            elif s < R:
                lane_step(name, path, lo, hi, s)
    sums(2, [(nm, lo, hi, 2) for nm, _, lo, hi in LANES], "g", ps)


_NC_CACHE = {}


def _get_nc():
    if "nc" not in _NC_CACHE:
        nc = bacc.Bacc("TRN2", target_bir_lowering=False, debug=False)
        CT_d = nc.dram_tensor("CT", [T2, T2 + 2 + B], BF16,
                              kind="ExternalInput")
        F_d = {}
        for name, path, lo, hi in LANES:
            F_d[name] = nc.dram_tensor(
                f"F{name}", [T2, R, hi - lo], BF16 if path == "A" else FP8,
                kind="ExternalInput")
        OUT_d = nc.dram_tensor("OUT", [4, F_COLS], F32, kind="ExternalOutput")
        with tile.TileContext(nc) as tc:
            _body(tc, OUT_d, CT_d, F_d)
        nc.compile()
        _NC_CACHE["nc"] = nc
    return _NC_CACHE["nc"]


def prepare_in_maps(feats, trans):
    feats = np.asarray(feats, dtype=np.float32)
    trans = np.asarray(trans, dtype=np.float32)
    assert feats.shape == (B, S, T2) and trans.shape == (T2, T2)

    with np.errstate(under="ignore", over="ignore"):
        W = np.exp(trans - SHIFT_W).T.astype(NPBF)       # [from, to]
        GE = np.ones((T2, 2), np.float32)
        GE[:, 1] = np.exp(trans[END, :])
        CTbase = np.concatenate([W.astype(np.float32), GE], axis=1)
        fexp = np.exp(feats.astype(np.float32) + 1.0)    # [B, S, T2]
    np.minimum(fexp, 416.0, out=fexp)
    F_full = np.ascontiguousarray(fexp.transpose(2, 1, 0))  # [T2, S, B]
    p0 = np.exp(trans[:, START] - SHIFT_W)[:, None] * F_full[:, 0, :]  # [T2,B]
    ones = np.ones((T2, B), np.float32)

    in_maps = []
    for k in range(NCORES):
        CT = np.concatenate([CTbase, p0 if k == 0 else ones],
                            axis=1).astype(NPBF)
        m = {"CT": CT}
        for name, path, lo, hi in LANES:
            buf = np.empty((T2, R, hi - lo), np.float32)
            for wi in range(lo // B, (hi + B - 1) // B):
                t0, w = WTAB[k][wi]
                ts = t0 - w
                clo, chi = max(lo, wi * B), min(hi, (wi + 1) * B)
                buf[:, :, clo - lo : chi - lo] = F_full[
                    :, ts : ts + R, clo - wi * B : chi - wi * B]
            m[f"F{name}"] = np.ascontiguousarray(
                buf.astype(NPBF if path == "A" else NPF8))
        in_maps.append(m)
    return in_maps


def postprocess(results):
    # OUT rows: 0 = delta(slot2), 1 = delta(slot5), 2 = gamma, 3 = w.y
    logZ = np.zeros(B, dtype=np.float64)
    for k, r in enumerate(results):
        out = r["OUT"].astype(np.float64)
        for wi in range(NWIN):
            lo = wi * B
            d2, d5 = out[0, lo : lo + B], out[1, lo : lo + B]
            g, wy = out[2, lo : lo + B], out[3, lo : lo + B]
            last = k == NCORES - 1 and wi == NWIN - 1
            logZ += np.log(wy) if last else np.log(g)
            if k == 0 and wi == 0:
                pass            # exact init, no delta
            elif k == 7 and wi == NWIN - 1:
                logZ -= np.log(d5)
            else:
                logZ -= np.log(d2)
    logZ += NET_SHIFT * S
    return logZ.astype(np.float32)


def run(feats, trans, trace=False, **spmd_kwargs):
    nc = _get_nc()
    in_maps = prepare_in_maps(feats, trans)
    res = run_bass_kernel_spmd(
        nc, in_maps, list(range(NCORES)), trace=trace, **spmd_kwargs
    )
    return postprocess(res.results), res


def kernel(feats, trans):
    out, _ = run(feats, trans, trace=False)
    return out


# revision 10
# speedup vs baseline: 1.1151x; 1.0952x over previous
"""Linear-chain CRF partition function (log Z) on 8 Trainium2 NeuronCores.

Exp-domain formulation: each step is q = W @ p (PE matmul, W = exp(trans-6).T)
followed by an elementwise multiply with f_t = exp(feat_t + 1).  The net
per-step shift is 5, accounted on the host (logZ += 5*S).

Time is split into 8 windows per batch column per core (64 windows of ~16
real steps per column globally).  All windows of a core advance in lockstep
through R=18 slots: w warmup slots (converge direction from a uniform start;
the transition matrix contracts direction error ~50x/step) then real steps.
Window scales are stitched on the host via colsum ratios (delta at the first
real slot, gamma at the end).

The 2048 in-flight columns (8 windows x 256 batch) are split across three
engine pipelines so every engine contributes:
  A (874 cols, bf16 feats): PE -> ACT copy (PSUM f32 -> SBUF bf16)
                            -> DVE 2x multiply (all-bf16).
  D (690 cols, fp8 feats):  PE -> DVE multiply (PSUM f32 x fp8).
  Z (484 cols, fp8 feats):  PE -> Pool (gpsimd) multiply.
Each lane owns its PSUM tile (concurrent readers need distinct tiles); the
stitching colsums reuse rows 0-1 of the lane's own PSUM tile so the four
drain pipelines run in parallel.
"""

import numpy as np
import ml_dtypes

import concourse.bacc as bacc
import concourse.bass as bass
import concourse.tile as tile
from concourse import mybir
from concourse._compat import with_exitstack
from concourse.bass_utils import run_bass_kernel_spmd

B, S, T2 = 256, 1024, 128
NCORES = 8
START, END = T2 - 1, T2 - 2
SHIFT_W = 6.0    # folded into the transition weights
SHIFT_F = -1.0   # folded into the features (f = exp(feat + 1))
NET_SHIFT = SHIFT_W + SHIFT_F  # 5 per step, host-corrected

R = 17                      # slots per window
NWIN = 8                    # windows per batch column per core
F_COLS = NWIN * B           # 2048 in-flight columns per core
CHUNKS = [4, 4, 3, 3, 3]  # feature-DMA chunking over the R slots
assert sum(CHUNKS) == R

# lanes: (name, path, lo, hi) over the 2048 column axis
import os
_SPLIT = os.environ.get("LANE_SPLIT", "440,440,880,0,288")
A1, A2, D1, D2, PC = (int(x) for x in _SPLIT.split(","))
assert A1 + A2 + D1 + D2 + PC == F_COLS
_B0, _B1, _B2, _B3, _B4 = (0, A1, A1 + A2, A1 + A2 + D1, A1 + A2 + D1 + D2)
LANES = [
    ("a2", "A", _B1, _B2),
    ("a1", "A", 0, _B1),
    ("d1", "D", _B2, _B3),
    ("d2", "D", _B3, _B4),
    ("p", "P", _B4, F_COLS),
]

# Per-core spans: core 0 has the exact-init window (18 real steps, w=0),
# core 7 ends with a 13-real window (w=5).  All other windows: 16 real, w=2.
# Column c of a core: window wi = c // 256, batch b = c % 256.
# All windows end their real span at slot R-1; warmup count w = R - len.
CORE_START = [1, 130, 258, 386, 514, 642, 770, 898]


def _win_table():
    """[core][wi] -> (t_first_real, w).  Slot s covers t = t_first_real-w+s."""
    tab = []
    for k in range(NCORES):
        rows = []
        t = CORE_START[k]
        for wi in range(NWIN):
            if k == 0 and wi == 0:
                ln, w = 17, 0
            elif k == 7 and wi == NWIN - 1:
                ln, w = 14, 3
            else:
                ln, w = 16, 1
            rows.append((t, w))
            t += ln
        tab.append(rows)
    assert t == S, (k, t)
    return tab


WTAB = _win_table()

BF16, F32, FP8 = mybir.dt.bfloat16, mybir.dt.float32, mybir.dt.float8e4
NPBF = ml_dtypes.bfloat16
NPF8 = ml_dtypes.float8_e4m3

SUM_ENG = {"a1": "scalar", "a2": "scalar", "d1": "scalar", "d2": "scalar", "p": "vector"}
SUM_ENG_G = {"a1": "scalar", "a2": "vector", "d1": "vector", "d2": "vector", "p": "scalar"}


@with_exitstack
def _body(ctx, tc, OUT_d, CT_d, F_d):
    nc = tc.nc
    const = ctx.enter_context(tc.tile_pool(name="const", bufs=1))
    ppool = ctx.enter_context(tc.tile_pool(name="p", bufs=4))
    fpool = ctx.enter_context(tc.tile_pool(name="f", bufs=1))
    qcpool = ctx.enter_context(tc.tile_pool(name="qc", bufs=2))
    opool = ctx.enter_context(tc.tile_pool(name="o", bufs=6))
    qpool = ctx.enter_context(
        tc.tile_pool(name="q", bufs=1, space=bass.MemorySpace.PSUM)
    )
    smpool = ctx.enter_context(
        tc.tile_pool(name="sm", bufs=1, space=bass.MemorySpace.PSUM)
    )
    sm_tiles = {}

    ct = const.tile([T2, T2 + 2 + B], BF16, tag="ct")
    nc.sync.dma_start(ct[:], CT_d[:])
    wmat = ct[:, 0:T2]
    ge = ct[:, T2 : T2 + 2]  # col0 = ones, col1 = exp(trans[END])

    # state init: ones everywhere (memset), exact p0 over core0-window0 (DMA;
    # other cores receive ones there too, data-driven)
    ps, qs, fts = {}, {}, {}
    meng = {"a1": nc.vector, "a2": nc.vector, "d1": nc.vector, "d2": nc.gpsimd, "p": nc.gpsimd}
    for name, path, lo, hi in LANES:
        n = hi - lo
        if n == 0:
            continue
        p = ppool.tile([T2, n], BF16, tag=f"p{name}", name=f"p_{name}_init")
        meng[name].memset(p[:], 1.0)
        ps[name] = p
        qs[name] = qpool.tile([T2, n], F32, tag=f"q{name}", name=f"q_{name}")
    nc.vector.tensor_copy(ps["a1"][:, 0:B], ct[:, T2 + 2 : T2 + 2 + B])

    bounds = list(np.cumsum([0] + CHUNKS))[:-1]
    # every chunk gets its own tile; all feature DMAs issue up front in need
    # order, so the serial DMA stream runs ahead of compute throughout
    chunk_tiles = {name: {} for name, _, _, _ in LANES}
    for ci in range(len(CHUNKS)):
        for name, path, lo, hi in LANES:
            n = hi - lo
            if n == 0:
                continue
            cs, b0 = CHUNKS[ci], bounds[ci]
            ft = fpool.tile([T2, cs, n], BF16 if path == "A" else FP8,
                            tag=f"f{name}c{ci}", name=f"f_{name}_c{ci}")
            nc.sync.dma_start(ft[:], F_d[name][:, b0 : b0 + cs, :])
            chunk_tiles[name][ci] = ft
    fts = {name: chunk_tiles[name][0] for name, _, lo, hi in LANES if hi > lo}
    issue_at = {}

    out_dmas = []

    def sums(row, lanes_cols, tag, src, sm_slot=None):
        # colsum rows of src[name] into scratch (or the lane's own q tile for
        # the final gammas), copy to SBUF, DMA out via the SP queue.
        for name, lo, hi, nr in lanes_cols:
            n = hi - lo
            if n == 0:
                continue
            if tag == "g":
                q = qs[name]
            else:
                if sm_slot not in sm_tiles:
                    sm_tiles[sm_slot] = smpool.tile(
                        [1, max(A1, D1) if sm_slot == 0 else max(PC, A2, D2, 1)], F32,
                        tag=f"sm{sm_slot}", name=f"sm{sm_slot}")
                q = sm_tiles[sm_slot][:, 0:n]
            for c0 in range(0, n, 512):
                c1 = min(c0 + 512, n)
                nc.tensor.matmul(q[0:nr, c0:c1], ge[:, 0:nr],
                                 src[name][:, c0:c1], start=True, stop=True)
            cp = opool.tile([2, n], F32, tag="cp", name=f"cp_{tag}_{name}")
            eng = getattr(nc, (SUM_ENG_G if tag == "g" else SUM_ENG)[name])
            if eng is nc.scalar:
                eng.copy(cp[0:nr, :], q[0:nr, :])
            else:
                eng.tensor_copy(cp[0:nr, :], q[0:nr, :])
            if tag == "g":
                nc.sync.dma_start(OUT_d[row : row + nr, lo:hi], cp[0:nr, :])
            else:
                out_dmas.append((row, nr, lo, hi, cp))

    def lane_mm(name, path, lo, hi, ls):
        n = hi - lo
        q, p = qs[name], ps[name]
        for c0 in range(0, n, 512):
            c1 = min(c0 + 512, n)
            nc.tensor.matmul(q[:, c0:c1], wmat[:], p[:, c0:c1], start=True,
                             stop=True)

    def lane_rest(name, path, lo, hi, ls):
        n = hi - lo
        ci = max(i for i, b in enumerate(bounds) if b <= ls)
        f = chunk_tiles[name][ci][:, ls - bounds[ci], :]
        q = qs[name]
        pn = ppool.tile([T2, n], BF16, tag=f"p{name}", name=f"p_{name}_{ls}")
        if path in ("A", "P"):
            qc = qcpool.tile([T2, n], BF16, tag=f"qc{name}",
                             name=f"qc_{name}_{ls}")
            nc.scalar.copy(qc[:], q[:])
            if path == "A":
                nc.vector.tensor_mul(pn[:], qc[:], f[:])
            else:
                nc.gpsimd.tensor_mul(pn[:], qc[:], f[:])
        else:
            nc.vector.tensor_mul(pn[:], q[:], f[:])
        ps[name] = pn

    for s in range(R + 1):
        if s == 1:
            state_s1 = dict(ps)
            sums(0, [("a1", 0, _B1, 1)], "d2", state_s1, sm_slot=0)
        if s == 2:
            # p runs a round behind: live ps["p"] is its post-step-0 state
            sums(0, [("p", _B4, F_COLS, 1)], "d2", ps, sm_slot=1)
        if s == 3:
            sums(0, [("d1", _B2, _B3, 1)], "d2", state_s1, sm_slot=0)
        if s == 4:
            # p post-step-2 state (w=3 window for core 7's last window)
            sums(1, [("p", _B4, F_COLS, 1)], "d5", ps, sm_slot=1)
        if s == 5:
            sums(0, [("a2", _B1, _B2, 1)], "d2", state_s1, sm_slot=0)
        if s == 6:
            sums(0, [("d2", _B3, _B4, 1)], "d2", state_s1, sm_slot=1)
        if s == 8:
            for row, nr, lo, hi, cp in out_dmas:
                nc.sync.dma_start(OUT_d[row : row + nr, lo:hi], cp[0:nr, :])
            out_dmas.clear()
        import os as _os
        mmo = _os.environ.get("MM_ORDER", "p,a2,a1,d1,d2").split(",")
        lmap = {nm: (nm2, pa, lo, hi) for nm2, pa, lo, hi in LANES for nm in [nm2]}
        for phase in ("mm", "rest"):
            order = mmo if phase == "mm" else [nm for nm, _, _, _ in LANES]
            for nm in order:
                name, path, lo, hi = lmap[nm]
                if hi == lo:
                    continue
                fn = lane_mm if phase == "mm" else lane_rest
                if name == "p":
                    if s >= 1:
                        fn(name, path, lo, hi, s - 1)
                elif s < R:
                    fn(name, path, lo, hi, s)
    sums(2, [(nm, lo, hi, 2) for nm, _, lo, hi in LANES], "g", ps)


_NC_CACHE = {}


def _get_nc():
    if "nc" not in _NC_CACHE:
        nc = bacc.Bacc("TRN2", target_bir_lowering=False, debug=False)
        CT_d = nc.dram_tensor("CT", [T2, T2 + 2 + B], BF16,
                              kind="ExternalInput")
        F_d = {}
        for name, path, lo, hi in LANES:
            if hi == lo:
                continue
            F_d[name] = nc.dram_tensor(
                f"F{name}", [T2, R, hi - lo], BF16 if path == "A" else FP8,
                kind="ExternalInput")
        OUT_d = nc.dram_tensor("OUT", [4, F_COLS], F32, kind="ExternalOutput")
        with tile.TileContext(nc) as tc:
            _body(tc, OUT_d, CT_d, F_d)
        nc.compile()
        _NC_CACHE["nc"] = nc
    return _NC_CACHE["nc"]


def prepare_in_maps(feats, trans):
    feats = np.asarray(feats, dtype=np.float32)
    trans = np.asarray(trans, dtype=np.float32)
    assert feats.shape == (B, S, T2) and trans.shape == (T2, T2)

    with np.errstate(under="ignore", over="ignore"):
        W = np.exp(trans - SHIFT_W).T.astype(NPBF)       # [from, to]
        GE = np.ones((T2, 2), np.float32)
        GE[:, 1] = np.exp(trans[END, :])
        CTbase = np.concatenate([W.astype(np.float32), GE], axis=1)
        fexp = np.exp(feats.astype(np.float32) + 1.0)    # [B, S, T2]
    np.minimum(fexp, 224.0, out=fexp)
    F_full = np.ascontiguousarray(fexp.transpose(2, 1, 0))  # [T2, S, B]
    p0 = np.exp(trans[:, START] - SHIFT_W)[:, None] * F_full[:, 0, :]  # [T2,B]
    ones = np.ones((T2, B), np.float32)

    in_maps = []
    for k in range(NCORES):
        CT = np.concatenate([CTbase, p0 if k == 0 else ones],
                            axis=1).astype(NPBF)
        m = {"CT": CT}
        for name, path, lo, hi in LANES:
            if hi == lo:
                continue
            buf = np.empty((T2, R, hi - lo), np.float32)
            for wi in range(lo // B, (hi + B - 1) // B):
                t0, w = WTAB[k][wi]
                ts = t0 - w
                clo, chi = max(lo, wi * B), min(hi, (wi + 1) * B)
                buf[:, :, clo - lo : chi - lo] = F_full[
                    :, ts : ts + R, clo - wi * B : chi - wi * B]
            m[f"F{name}"] = np.ascontiguousarray(
                buf.astype(NPBF if path == "A" else NPF8))
        in_maps.append(m)
    return in_maps


def postprocess(results):
    # OUT rows: 0 = delta(slot2), 1 = delta(slot5), 2 = gamma, 3 = w.y
    logZ = np.zeros(B, dtype=np.float64)
    for k, r in enumerate(results):
        out = r["OUT"].astype(np.float64)
        for wi in range(NWIN):
            lo = wi * B
            d2, d5 = out[0, lo : lo + B], out[1, lo : lo + B]
            g, wy = out[2, lo : lo + B], out[3, lo : lo + B]
            last = k == NCORES - 1 and wi == NWIN - 1
            logZ += np.log(wy) if last else np.log(g)
            if k == 0 and wi == 0:
                pass            # exact init, no delta
            elif k == 7 and wi == NWIN - 1:
                logZ -= np.log(d5)
            else:
                logZ -= np.log(d2)
    logZ += NET_SHIFT * S
    return logZ.astype(np.float32)


def run(feats, trans, trace=False, **spmd_kwargs):
    nc = _get_nc()
    in_maps = prepare_in_maps(feats, trans)
    res = run_bass_kernel_spmd(
        nc, in_maps, list(range(NCORES)), trace=trace, **spmd_kwargs
    )
    return postprocess(res.results), res


def kernel(feats, trans):
    out, _ = run(feats, trans, trace=False)
    return out
